# revision 61
# baseline (speedup 1.0000x reference)
"""HGT layer (heterogeneous graph transformer) on 8 trn2 NeuronCores.

Strategy (dst-sharded, fully on-device message passing):
  * Edges of each relation are sorted by dst on host and sharded across the 8
    cores by dst range (core c owns dst rows [c*3750, (c+1)*3750) of the
    relevant node type). No collectives are needed: node features h0/h1 are
    replicated (inputs), per-edge K/V projections are computed on device from
    gathered h rows, and Q is a small per-core table (own dst rows only).
  * Per relation, edges are packed into "blocks": <=128 consecutive dsts and
    <=640 edges (5 chunks of 128). Per block we:
      - load Q_b = qtab[d_lo:d_lo+128] via a dynamic-offset DMA (d_lo read
        from a per-core input with value_load)
      - per chunk: 2 matmuls against [Wk_eff | Wv_eff] -> kv PSUM [128e, 512]
      - per-edge q rows via a one-hot expand matmul: qg = stT_chunk^T @ Q_b
      - score s = per-head sum(qg * k) (DVE mul + reduce), ex = exp(s) (ACT)
      - rhs = [v * ex_broadcast | ex] bf16
      - banded segment-sum: matmul(U += st_chunk^T @ rhs) accumulating in PSUM
    After 5 chunks: t = U[:, :256] / (U[:, 256:264] + eps) per head.
  * The 0.5 cross-relation mean factor is folded into Wv_eff for relations
    0 and 2 host-side. Softmax max-subtraction is skipped (scores ~ N(0,1));
    the dst-constant score bias (q . bk_eff) cancels in the per-dst softmax,
    so bk is dropped exactly. bv_eff is folded in after normalization; bq is
    added into the q table.
  * The q table (own dst rows) is built on device: h rows are loaded with a
    DMA transpose (consecutive rows, no gather) and projected with Wq.
  * n1 receives rel0 and rel2 with shared block boundaries; t1 = t0' + t2'
    (0.5 pre-folded).
  * Output: t is PE-transposed, matmul'd with Wa, and combined with the
    pre-scaled skip rows (host-packed h*(1-alpha) + alpha*ba).
Outputs are written in packed-block order; host unpacks to [2, 30000, 256].
"""

import math

import numpy as np
import ml_dtypes

import concourse.bass as bass
import concourse.bacc as bacc
import concourse.tile as tile
from concourse import mybir
from concourse.bass_utils import run_bass_kernel_spmd
from concourse.masks import make_identity

BF16 = ml_dtypes.bfloat16
FP8 = ml_dtypes.float8_e4m3

N = 30000
NPAD = 30720               # h tables padded so per-core q windows stay in range
D = 256
H = 8
DK = 32
E = 160000
NCORES = 8
RPC = N // NCORES          # dst rows per core
BLK = 640                  # max edges per block
CPB = BLK // 128           # chunks per block (5)
GRP = 1                    # blocks per gather group (transposed dma_gather
                           # crashes on HW above 640 idxs, so no grouping)
GIDX = GRP * BLK           # indices per gather
ICOLS = BLK // 16          # idx columns per block (40)
QTR = 3968                 # q table rows (covers d_lo+128 for any block)

SRC_OF_REL = (0, 1, 1)     # node type of src per relation
DST_OF_REL = (1, 0, 1)     # node type of dst per relation

_cache = {}


# ----------------------------------------------------------------------------
# Host preprocessing
# ----------------------------------------------------------------------------

def _block_diag(mats):
    # mats: [H, DK, DK] -> [D, D] block diagonal
    out = np.zeros((H * mats.shape[1], H * mats.shape[2]), np.float32)
    for h in range(mats.shape[0]):
        out[h * DK:(h + 1) * DK, h * DK:(h + 1) * DK] = mats[h]
    return out


def _wrap_idx(idx_groups, gidx=None):
    """idx_groups: list of int arrays each of length gidx. Returns
    [128, len*gidx//16] int16 in the 16-partition-wrapped dma_gather layout."""
    gidx = GIDX if gidx is None else gidx
    ncol = len(idx_groups) * (gidx // 16)
    out = np.zeros((128, ncol), np.int16)
    for g, arr in enumerate(idx_groups):
        w = np.asarray(arr, np.int16).reshape(gidx // 16, 16).T
        out[:, g * (gidx // 16):(g + 1) * (gidx // 16)] = np.tile(w, (8, 1))
    return out


def _pack_blocks(seg_counts_list):
    """Greedy-pack consecutive dsts into blocks.
    seg_counts_list: list of per-dst edge counts arrays (all same length RPC);
    a block must satisfy <=128 dsts and <=BLK edges in EVERY relation given.
    Returns list of (d_lo, nd)."""
    n = len(seg_counts_list[0])
    blocks = []
    i = 0
    while i < n:
        d0 = i
        e = [0] * len(seg_counts_list)
        while i < n and (i - d0) < 128:
            ok = all(e[k] + seg_counts_list[k][i] <= BLK
                     for k in range(len(seg_counts_list)))
            if not ok:
                break
            for k in range(len(seg_counts_list)):
                e[k] += seg_counts_list[k][i]
            i += 1
        assert i > d0, "single dst segment exceeds BLK edges"
        blocks.append((d0, i - d0))
    return blocks


def prep(inputs):
    h0 = np.asarray(inputs['h0'], np.float32)
    h1 = np.asarray(inputs['h1'], np.float32)
    Wk = np.asarray(inputs['Wk'], np.float32)
    bk = np.asarray(inputs['bk'], np.float32)
    Wq = np.asarray(inputs['Wq'], np.float32)
    bq = np.asarray(inputs['bq'], np.float32)
    Wv = np.asarray(inputs['Wv'], np.float32)
    bv = np.asarray(inputs['bv'], np.float32)
    Wa = np.asarray(inputs['Wa'], np.float32)
    ba = np.asarray(inputs['ba'], np.float32)
    rel_att = np.asarray(inputs['rel_att'], np.float32)
    rel_msg = np.asarray(inputs['rel_msg'], np.float32)
    rel_pri = np.asarray(inputs['rel_pri'], np.float32)
    skip = np.asarray(inputs['skip'], np.float32)

    alpha = 1.0 / (1.0 + np.exp(-skip))          # [2]
    hs = [h0, h1]

    # effective projections (att/msg/pri folded); bk dropped (cancels in the
    # per-dst softmax). The 0.5 cross-relation mean is folded into Wv/bv of
    # relations 0 and 2.
    Wk_eff, Wv_eff, bv_eff = [], [], []
    for r in range(3):
        st = SRC_OF_REL[r]
        half = 0.5 if r != 1 else 1.0
        A = _block_diag(rel_att[r])
        M = _block_diag(rel_msg[r])
        scale = np.repeat(rel_pri[r] / math.sqrt(DK), DK)  # [256]
        Wk_eff.append((Wk[st] @ A) * scale[None, :])
        Wv_eff.append((Wv[st] @ M) * half)
        bv_eff.append((bv[st] @ M) * half)

    # edge sorting by dst
    edges = []
    for r, (skey, dkey) in enumerate((('src0', 'dst0'), ('src1', 'dst1'),
                                      ('src2', 'dst2'))):
        src = np.asarray(inputs[skey], np.int64)
        dst = np.asarray(inputs[dkey], np.int64)
        order = np.argsort(dst, kind='stable')
        ssrc = src[order]
        sdst = dst[order]
        counts = np.bincount(dst, minlength=N)
        starts = np.zeros(N + 1, np.int64)
        np.cumsum(counts, out=starts[1:])
        edges.append((ssrc, sdst, counts, starts))

    # per-core packing
    per_core = []
    for c in range(NCORES):
        lo = c * RPC
        cnt1 = edges[1][2][lo:lo + RPC]
        blocks0 = _pack_blocks([cnt1])                       # n0 side (rel1)
        cnt0 = edges[0][2][lo:lo + RPC]
        cnt2 = edges[2][2][lo:lo + RPC]
        blocks1 = _pack_blocks([cnt0, cnt2])                 # n1 side (rel0+2)
        per_core.append((blocks0, blocks1))

    NB0 = max(len(pc[0]) for pc in per_core)
    NB1 = max(len(pc[1]) for pc in per_core)
    NB0 += NB0 % 2
    NB1 += NB1 % 2

    # bf16 replicated tables (padded rows are zero)
    h0b = np.zeros((NPAD, D), BF16)
    h0b[:N] = h0.astype(BF16)
    h1b = np.zeros((NPAD, D), BF16)
    h1b[:N] = h1.astype(BF16)

    # weights, chunked for matmul rhs
    wq_t = np.stack([Wq[t].reshape(2, 128, D) for t in range(2)]).astype(BF16)
    wkv_t = np.stack([
        np.concatenate([Wk_eff[r], Wv_eff[r]], axis=1).reshape(2, 128, 2 * D)
        for r in range(3)]).astype(BF16)
    wa_t = np.stack([Wa[t].reshape(2, 128, D) for t in range(2)]).astype(BF16)
    bq_t = bq.copy()                                          # [2, 256] f32
    bv_t = np.stack([bv_eff[1], bv_eff[0] + bv_eff[2]])       # [2, 256]
    use_bv = bool(np.abs(bv_t).max() > 0)

    in_maps = []
    unpack = []
    for c in range(NCORES):
        lo = c * RPC
        blocks0, blocks1 = per_core[c]
        m = {
            'h0b': h0b, 'h1b': h1b,
            'hq0': h0b[lo:lo + QTR].copy(), 'hq1': h1b[lo:lo + QTR].copy(),
            'wq': wq_t, 'wkv': wkv_t, 'wa': wa_t,
            'bq2': bq_t, 'bvt': bv_t.astype(np.float32),
        }
        # per-block q-window offsets (d_lo), side0 blocks then side1 blocks
        dlo = np.zeros((1, NB0 + NB1), np.int32)
        for i, (d_lo, nd) in enumerate(blocks0):
            dlo[0, i] = d_lo
        for i, (d_lo, nd) in enumerate(blocks1):
            dlo[0, NB0 + i] = d_lo
        m['dlo'] = dlo

        # per relation edge data
        rel_blocks = {0: blocks1, 1: blocks0, 2: blocks1}
        rel_nb = {0: NB1, 1: NB0, 2: NB1}
        for r in range(3):
            ssrc, sdst, counts, starts = edges[r]
            blocks = rel_blocks[r]
            nb = rel_nb[r]
            sidx = np.zeros((nb * BLK,), np.int64)
            st = np.zeros((nb, 128, CPB * 128), FP8)
            stT = np.zeros((nb, 128, CPB * 128), FP8)
            for b, (d_lo, nd) in enumerate(blocks):
                e0 = starts[lo + d_lo]
                e1 = starts[lo + d_lo + nd]
                ne = e1 - e0
                assert ne <= BLK
                if ne == 0:
                    continue
                sidx[b * BLK: b * BLK + ne] = ssrc[e0:e1]
                slot = (sdst[e0:e1] - (lo + d_lo)).astype(np.int64)
                j = np.arange(ne)
                st[b, j % 128, (j // 128) * 128 + slot] = 1.0
                stT[b, slot, (j // 128) * 128 + (j % 128)] = 1.0
            groups = [sidx[g * GIDX:(g + 1) * GIDX]
                      for g in range(nb // GRP)]
            m[f'sidx{r}'] = _wrap_idx(groups)
            m[f'st{r}'] = st
            m[f'stT{r}'] = stT

        # skip rows, packed; pre-scaled: h*(1-a) + a*ba
        hsk = np.zeros(((NB0 + NB1) * 128, D), BF16)
        for i, (d_lo, nd) in enumerate(blocks0):
            hsk[i * 128: i * 128 + nd] = (hs[0][lo + d_lo: lo + d_lo + nd]
                                          * (1 - alpha[0]) + alpha[0] * ba[0])
        for i, (d_lo, nd) in enumerate(blocks1):
            hsk[(NB0 + i) * 128:(NB0 + i) * 128 + nd] = (
                hs[1][lo + d_lo: lo + d_lo + nd] * (1 - alpha[1])
                + alpha[1] * ba[1])
        m['hsk'] = hsk
        in_maps.append(m)
        unpack.append((blocks0, blocks1))

    # per-block-index qtab write coverage (max over cores): the qb DMA of
    # block b only needs qtab rows < (jm[b]+1)*128 to be written.
    jm0 = [0] * NB0
    jm1 = [0] * NB1
    for c in range(NCORES):
        blocks0, blocks1 = per_core[c]
        for i, (d_lo, nd) in enumerate(blocks0):
            jm0[i] = max(jm0[i], (d_lo + 127) // 128)
        for i, (d_lo, nd) in enumerate(blocks1):
            jm1[i] = max(jm1[i], (d_lo + 127) // 128)
    meta = dict(NB0=NB0, NB1=NB1, alpha=(float(alpha[0]), float(alpha[1])),
                use_bv=use_bv, jm0=tuple(jm0), jm1=tuple(jm1))
    return in_maps, unpack, meta


# ----------------------------------------------------------------------------
# Device program
# ----------------------------------------------------------------------------

def build_program(NB0, NB1, alpha, use_bv, jm0, jm1):
    fp32 = mybir.dt.float32
    bf16 = mybir.dt.bfloat16
    fp8 = mybir.dt.float8e4
    i16 = mybir.dt.int16
    i32 = mybir.dt.int32
    nc = bacc.Bacc('TRN2', target_bir_lowering=False, debug=False,
                   num_devices=NCORES,
                   dynamic_dma_scratch_size=65536)

    h0b = nc.dram_tensor('h0b', [NPAD, D], bf16, kind='ExternalInput')
    h1b = nc.dram_tensor('h1b', [NPAD, D], bf16, kind='ExternalInput')
    hq = [nc.dram_tensor(f'hq{t}', [QTR, D], bf16, kind='ExternalInput')
          for t in range(2)]
    wq = nc.dram_tensor('wq', [2, 2, 128, D], bf16, kind='ExternalInput')
    wkv = nc.dram_tensor('wkv', [3, 2, 128, 2 * D], bf16, kind='ExternalInput')
    wa = nc.dram_tensor('wa', [2, 2, 128, D], bf16, kind='ExternalInput')
    bq2 = nc.dram_tensor('bq2', [2, D], fp32, kind='ExternalInput')
    bvt = nc.dram_tensor('bvt', [2, D], fp32, kind='ExternalInput')
    dlo_t = nc.dram_tensor('dlo', [1, NB0 + NB1], i32, kind='ExternalInput')
    sidx = [nc.dram_tensor(f'sidx{r}', [128, (NB1 if r != 1 else NB0) * ICOLS],
                           i16, kind='ExternalInput') for r in range(3)]
    st = [nc.dram_tensor(f'st{r}', [(NB1 if r != 1 else NB0), 128, CPB * 128],
                         fp8, kind='ExternalInput') for r in range(3)]
    stT = [nc.dram_tensor(f'stT{r}', [(NB1 if r != 1 else NB0), 128,
                                      CPB * 128],
                          fp8, kind='ExternalInput') for r in range(3)]
    hsk = nc.dram_tensor('hsk', [(NB0 + NB1) * 128, D], bf16,
                         kind='ExternalInput')
    outp = nc.dram_tensor('outp', [(NB0 + NB1) * 128, D], bf16,
                          kind='ExternalOutput')
    qtab = [nc.dram_tensor(f'q{t}tab', [QTR, D], bf16, kind='Internal')
            for t in range(2)]

    htab = [h0b, h1b]
    REL_NB = {0: NB1, 1: NB0, 2: NB1}

    with tile.TileContext(nc) as tc:
        with (
            tc.tile_pool(name='singles', bufs=1) as singles,
            tc.tile_pool(name='gpool', bufs=10) as gpool,
            tc.tile_pool(name='spool', bufs=4) as spool,
            tc.tile_pool(name='work', bufs=12) as work,
            tc.tile_pool(name='opool', bufs=3) as opool,
            tc.tile_pool(name='psA', bufs=2, space='PSUM') as psA,
            tc.tile_pool(name='psQ', bufs=2, space='PSUM') as psQ,
            tc.tile_pool(name='psU', bufs=1, space='PSUM') as psU,
            tc.tile_pool(name='psO', bufs=1, space='PSUM') as psO,
        ):
            from concourse import library_config
            from concourse.tile import add_dep_helper
            nc.gpsimd.load_library(library_config.mlp)
            gidx_reg = nc.gpsimd.to_reg(GIDX)
            qdma_hist = []

            ident = singles.tile([128, 128], bf16)
            make_identity(nc, ident[:])

            # resident weights
            wq_sb = singles.tile([128, 2, 2, D], bf16)
            nc.sync.dma_start(out=wq_sb[:],
                              in_=wq[:].rearrange('a b p n -> p a b n'))
            wkv_sb = singles.tile([128, 3, 2, 2 * D], bf16)
            nc.sync.dma_start(out=wkv_sb[:],
                              in_=wkv[:].rearrange('a b p n -> p a b n'))
            wa_sb = singles.tile([128, 2, 2, D], bf16)
            nc.sync.dma_start(out=wa_sb[:],
                              in_=wa[:].rearrange('a b p n -> p a b n'))
            bq_sb = singles.tile([128, 2, D], fp32)
            bv_sb = singles.tile([128, 2, D], fp32)
            for t in range(2):
                src = bq2[t:t + 1, :]
                nc.sync.dma_start(out=bq_sb[:, t, :],
                                  in_=src.to_broadcast([128, D]))
                src = bvt[t:t + 1, :]
                nc.sync.dma_start(out=bv_sb[:, t, :],
                                  in_=src.to_broadcast([128, D]))

            # resident indices + per-block q-window offsets
            sidx_sb = []
            for r in range(3):
                t1 = singles.tile([128, REL_NB[r] * ICOLS], i16,
                                  tag=f'sidx{r}')
                nc.sync.dma_start(out=t1[:], in_=sidx[r][:])
                sidx_sb.append(t1)
            dlo_sb = singles.tile([1, NB0 + NB1], i32, tag='dlo')
            nc.sync.dma_start(out=dlo_sb[:], in_=dlo_t[:])

            # t-store for rel0 results (n1 side)
            tstore = singles.tile([128, NB1, D], bf16)

            # own h rows, DMA-transposed (lhsT layout) for the q projection
            gth_t = []
            for t in range(2):
                g = singles.tile([128, 2, QTR], bf16, tag=f'gth{t}')
                for cc in range(2):
                    nc.scalar.dma_start_transpose(
                        out=g[:, cc, :],
                        in_=hq[t][:, cc * 128:(cc + 1) * 128])
                gth_t.append(g)

            # ---------------- phase: q tables ----------------
            # No barrier afterwards: each per-block qb DMA carries explicit
            # dep edges on this side's qtab writes. Alternate psA/psQ tiles
            # so the projection pipeline is >1 deep.
            qtab_dmas = [[], []]
            for t in range(2):
                for j in range(QTR // 128):
                    if j % 2 == 0:
                        qp = psA.tile([128, 2, 2 * D], fp32, tag='kv')
                    else:
                        qp = psQ.tile([128, 2, D], fp32, tag='qg')
                    for cc in range(2):
                        nc.tensor.matmul(
                            out=qp[:, 0, :D],
                            lhsT=gth_t[t][:, cc, j * 128:(j + 1) * 128],
                            rhs=wq_sb[:, t, cc, :],
                            start=(cc == 0), stop=(cc == 1))
                    qs = work.tile([128, D], bf16, tag='qs')
                    nc.vector.tensor_add(qs[:], qp[:, 0, :D],
                                         bq_sb[:, t, :])
                    dma = nc.sync.dma_start(
                        out=qtab[t][j * 128:(j + 1) * 128, :], in_=qs[:])
                    qtab_dmas[t].append(dma)

            # ---------------- relation passes ----------------
            def rel_pass(r, mode):
                # mode: 'out' (rel1), 'store' (rel0), 'combine' (rel2)
                nb = REL_NB[r]
                side = DST_OF_REL[r]
                out_off = 0 if side == 0 else NB0
                scol = 0 if side == 0 else NB0
                for g in range(nb // GRP):
                    sg = gpool.tile([128, 2, GIDX], bf16, tag='gs')
                    nc.gpsimd.dma_gather(
                        out_ap=sg[:], in_ap=htab[SRC_OF_REL[r]][:],
                        idxs_ap=sidx_sb[r][:, g * (GIDX // 16):
                                           (g + 1) * (GIDX // 16)],
                        num_idxs=GIDX, num_idxs_reg=gidx_reg,
                        elem_size=D, transpose=True)
                    for bb in range(GRP):
                        b = g * GRP + bb
                        # block q rows (d_lo..d_lo+127) via dynamic-offset
                        # DMA; an explicit dep on the 4-back qb DMA bounds
                        # sync-engine register liveness.
                        dreg = nc.sync.alloc_register(f'dlo_{r}_{b}')
                        ld = nc.sync.reg_load(
                            dreg, dlo_sb[0:1, scol + b: scol + b + 1])
                        if len(qdma_hist) >= 4:
                            add_dep_helper(ld.ins, qdma_hist[-4].ins,
                                           sync=True,
                                           reason='bound dlo reg liveness')
                        dval = nc.sync.snap(dreg, donate=True, min_val=0,
                                            max_val=QTR - 128)
                        qb = opool.tile([128, D], bf16, tag='qb')
                        qdma = nc.sync.dma_start(
                            out=qb[:], in_=qtab[side][bass.ds(dval, 128), :])
                        jm = (jm0 if side == 0 else jm1)[b]
                        for wdma in qtab_dmas[side][:jm + 1]:
                            add_dep_helper(qdma.ins, wdma.ins, sync=True,
                                           reason='qtab RAW')
                        qdma_hist.append(qdma)
                        stt = spool.tile([128, CPB * 128], fp8, tag='st')
                        nc.sync.dma_start(out=stt[:], in_=st[r][b])
                        sttT = spool.tile([128, CPB * 128], fp8, tag='stT')
                        nc.sync.dma_start(out=sttT[:], in_=stT[r][b])
                        u = psU.tile([128, 264], fp32, tag='u')
                        for pair in ((0, 1), (2, 3), (4,)):
                            npair = len(pair)
                            kv = psA.tile([128, 2, 2 * D], fp32, tag='kv')
                            qg_ps = psQ.tile([128, 2, D], fp32, tag='qg')
                            for i, j in enumerate(pair):
                                ci = bb * CPB + j
                                for cc in range(2):
                                    nc.tensor.matmul(
                                        out=kv[:, i, :],
                                        lhsT=sg[:, cc,
                                                ci * 128:(ci + 1) * 128],
                                        rhs=wkv_sb[:, r, cc, :],
                                        start=(cc == 0), stop=(cc == 1))
                                nc.tensor.matmul(
                                    out=qg_ps[:, i, :],
                                    lhsT=sttT[:, j * 128:(j + 1) * 128],
                                    rhs=qb[:], start=True, stop=True)
                            qg = work.tile([128, 2, D], bf16, tag='qgs')
                            nc.scalar.activation(
                                qg[:, :npair, :], qg_ps[:, :npair, :],
                                mybir.ActivationFunctionType.Copy)
                            p = work.tile([128, 2, D], bf16, tag='p')
                            nc.vector.tensor_mul(p[:, :npair, :],
                                                 qg[:, :npair, :],
                                                 kv[:, :npair, :D])
                            s8 = work.tile([128, 2, H], fp32, tag='s8')
                            nc.vector.reduce_sum(
                                s8[:, :npair, :],
                                p[:, :npair, :].rearrange(
                                    'p a (h d) -> p a h d', d=DK),
                                axis=mybir.AxisListType.X)
                            ex = work.tile([128, 2, H], fp32, tag='ex')
                            nc.scalar.activation(
                                ex[:, :npair, :], s8[:, :npair, :],
                                mybir.ActivationFunctionType.Exp)
                            rhs = work.tile([128, 2, 264], bf16, tag='rhs')
                            exb = ex[:, :npair, :]
                            exb = bass.AP(tensor=exb.tensor, offset=exb.offset,
                                          ap=[*exb.ap, [0, DK]])
                            nc.vector.tensor_mul(
                                rhs[:, :npair, :D].rearrange(
                                    'p a (h d) -> p a h d', d=DK),
                                kv[:, :npair, D:].rearrange(
                                    'p a (h d) -> p a h d', d=DK),
                                exb)
                            nc.scalar.activation(
                                rhs[:, :npair, D:D + H], ex[:, :npair, :],
                                mybir.ActivationFunctionType.Copy)
                            for i, j in enumerate(pair):
                                nc.tensor.matmul(
                                    out=u[:],
                                    lhsT=stt[:, j * 128:(j + 1) * 128],
                                    rhs=rhs[:, i, :],
                                    start=(j == 0), stop=(j == CPB - 1))
                        # normalize
                        rcp = work.tile([128, H], fp32, tag='rcp')
                        nc.vector.tensor_scalar_add(rcp[:], u[:, D:D + H],
                                                    1e-20)
                        nc.vector.reciprocal(rcp[:], rcp[:])
                        rcpb = rcp[:]
                        rcpb = bass.AP(tensor=rcpb.tensor, offset=rcpb.offset,
                                       ap=[*rcpb.ap, [0, DK]])
                        if mode == 'store':
                            nc.vector.tensor_mul(
                                tstore[:, b, :].rearrange(
                                    'p (h d) -> p h d', d=DK),
                                u[:, :D].rearrange('p (h d) -> p h d', d=DK),
                                rcpb)
                            continue
                        t_sb = opool.tile([128, D], bf16, tag='t')
                        nc.vector.tensor_mul(
                            t_sb[:].rearrange('p (h d) -> p h d', d=DK),
                            u[:, :D].rearrange('p (h d) -> p h d', d=DK),
                            rcpb)
                        if mode == 'combine':
                            nc.vector.tensor_add(t_sb[:], t_sb[:],
                                                 tstore[:, b, :])
                        if use_bv:
                            nc.vector.tensor_add(t_sb[:], t_sb[:],
                                                 bv_sb[:, side, :])
                        # output: transpose, matmul Wa, skip-combine. om and
                        # the transpose scratch share one PSUM bank.
                        tts = opool.tile([128, 2, 128], bf16, tag='tts')
                        omtp = psO.tile([128, 384], fp32, tag='omtp')
                        om = omtp[:, :D]
                        tp = omtp[:, D:D + 64].bitcast(bf16)
                        for cc in range(2):
                            nc.tensor.transpose(
                                tp, t_sb[:, cc * 128:(cc + 1) * 128],
                                ident[:])
                            nc.scalar.activation(
                                tts[:, cc, :], tp,
                                mybir.ActivationFunctionType.Copy)
                        for cc in range(2):
                            nc.tensor.matmul(
                                out=om, lhsT=tts[:, cc, :],
                                rhs=wa_sb[:, side, cc, :],
                                start=(cc == 0), stop=(cc == 1))
                        hs_t = opool.tile([128, D], bf16, tag='hs')
                        row = (out_off + b) * 128
                        nc.sync.dma_start(out=hs_t[:],
                                          in_=hsk[row:row + 128, :])
                        o_sb = opool.tile([128, D], bf16, tag='o')
                        nc.vector.scalar_tensor_tensor(
                            out=o_sb[:], in0=om, scalar=alpha[side],
                            in1=hs_t[:], op0=mybir.AluOpType.mult,
                            op1=mybir.AluOpType.add)
                        nc.sync.dma_start(out=outp[row:row + 128, :],
                                          in_=o_sb[:])

            rel_pass(1, 'out')
            rel_pass(0, 'store')
            rel_pass(2, 'combine')

    nc.compile()
    return nc


# ----------------------------------------------------------------------------
# Entry point
# ----------------------------------------------------------------------------

def _run(inputs, trace=False):
    in_maps, unpack, meta = prep(inputs)
    key = (meta['NB0'], meta['NB1'], meta['alpha'], meta['use_bv'],
           meta['jm0'], meta['jm1'])
    if key not in _cache:
        _cache[key] = build_program(meta['NB0'], meta['NB1'], meta['alpha'],
                                    meta['use_bv'], meta['jm0'], meta['jm1'])
    nc = _cache[key]
    res = run_bass_kernel_spmd(nc, in_maps, core_ids=list(range(NCORES)),
                               trace=trace)
    NB0 = meta['NB0']
    out = np.zeros((2, N, D), np.float32)
    for c in range(NCORES):
        lo = c * RPC
        op = res.results[c]['outp']
        blocks0, blocks1 = unpack[c]
        for i, (d_lo, nd) in enumerate(blocks0):
            out[0, lo + d_lo: lo + d_lo + nd] = op[i * 128: i * 128 + nd]
        for i, (d_lo, nd) in enumerate(blocks1):
            out[1, lo + d_lo: lo + d_lo + nd] = op[(NB0 + i) * 128:
                                                   (NB0 + i) * 128 + nd]
    return out, res


def kernel(**inputs):
    out, _ = _run(inputs, trace=False)
    return out


# revision 62
# speedup vs baseline: 1.0282x; 1.0282x over previous
"""HGT layer (heterogeneous graph transformer) on 8 trn2 NeuronCores.

Strategy (dst-sharded, fully on-device message passing):
  * Edges of each relation are sorted by dst on host and sharded across the 8
    cores by dst range (core c owns dst rows [c*3750, (c+1)*3750) of the
    relevant node type). No collectives are needed: node features h0/h1 are
    replicated (inputs), per-edge K/V projections are computed on device from
    gathered h rows, and Q is a small per-core table (own dst rows only).
  * Per relation, edges are packed into "blocks": <=128 consecutive dsts and
    <=640 edges (5 chunks of 128). Per block we:
      - load Q_b = qtab[d_lo:d_lo+128] via a dynamic-offset DMA (d_lo read
        from a per-core input with value_load)
      - per chunk: 2 matmuls against [Wk_eff | Wv_eff] -> kv PSUM [128e, 512]
      - per-edge q rows via a one-hot expand matmul: qg = stT_chunk^T @ Q_b
      - score s = per-head sum(qg * k) (DVE mul + reduce), ex = exp(s) (ACT)
      - rhs = [v * ex_broadcast | ex] bf16
      - banded segment-sum: matmul(U += st_chunk^T @ rhs) accumulating in PSUM
    After 5 chunks: t = U[:, :256] / (U[:, 256:264] + eps) per head.
  * The 0.5 cross-relation mean factor is folded into Wv_eff for relations
    0 and 2 host-side. Softmax max-subtraction is skipped (scores ~ N(0,1));
    the dst-constant score bias (q . bk_eff) cancels in the per-dst softmax,
    so bk is dropped exactly. bv_eff is folded in after normalization; bq is
    added into the q table.
  * The q table (own dst rows) is built on device: h rows are loaded with a
    DMA transpose (consecutive rows, no gather) and projected with Wq.
  * n1 receives rel0 and rel2 with shared block boundaries; t1 = t0' + t2'
    (0.5 pre-folded).
  * Output: t is PE-transposed, matmul'd with Wa, and combined with the
    pre-scaled skip rows (host-packed h*(1-alpha) + alpha*ba).
Outputs are written in packed-block order; host unpacks to [2, 30000, 256].
"""

import math

import numpy as np
import ml_dtypes

import concourse.bass as bass
import concourse.bacc as bacc
import concourse.tile as tile
from concourse import mybir
from concourse.bass_utils import run_bass_kernel_spmd
from concourse.masks import make_identity

BF16 = ml_dtypes.bfloat16
FP8 = ml_dtypes.float8_e4m3

N = 30000
NPAD = 30720               # h tables padded so per-core q windows stay in range
D = 256
H = 8
DK = 32
E = 160000
NCORES = 8
RPC = N // NCORES          # dst rows per core
BLK = 640                  # max edges per block
CPB = BLK // 128           # chunks per block (5)
GRP = 1                    # blocks per gather group (transposed dma_gather
                           # crashes on HW above 640 idxs, so no grouping)
GIDX = GRP * BLK           # indices per gather
ICOLS = BLK // 16          # idx columns per block (40)
QTR = 3968                 # q table rows (covers d_lo+128 for any block)

SRC_OF_REL = (0, 1, 1)     # node type of src per relation
DST_OF_REL = (1, 0, 1)     # node type of dst per relation

_cache = {}


# ----------------------------------------------------------------------------
# Host preprocessing
# ----------------------------------------------------------------------------

def _block_diag(mats):
    # mats: [H, DK, DK] -> [D, D] block diagonal
    out = np.zeros((H * mats.shape[1], H * mats.shape[2]), np.float32)
    for h in range(mats.shape[0]):
        out[h * DK:(h + 1) * DK, h * DK:(h + 1) * DK] = mats[h]
    return out


def _wrap_idx(idx_groups, gidx=None):
    """idx_groups: list of int arrays each of length gidx. Returns
    [128, len*gidx//16] int16 in the 16-partition-wrapped dma_gather layout."""
    gidx = GIDX if gidx is None else gidx
    ncol = len(idx_groups) * (gidx // 16)
    out = np.zeros((128, ncol), np.int16)
    for g, arr in enumerate(idx_groups):
        w = np.asarray(arr, np.int16).reshape(gidx // 16, 16).T
        out[:, g * (gidx // 16):(g + 1) * (gidx // 16)] = np.tile(w, (8, 1))
    return out


def _pack_blocks(seg_counts_list):
    """Greedy-pack consecutive dsts into blocks.
    seg_counts_list: list of per-dst edge counts arrays (all same length RPC);
    a block must satisfy <=128 dsts and <=BLK edges in EVERY relation given.
    Returns list of (d_lo, nd)."""
    n = len(seg_counts_list[0])
    blocks = []
    i = 0
    while i < n:
        d0 = i
        e = [0] * len(seg_counts_list)
        while i < n and (i - d0) < 128:
            ok = all(e[k] + seg_counts_list[k][i] <= BLK
                     for k in range(len(seg_counts_list)))
            if not ok:
                break
            for k in range(len(seg_counts_list)):
                e[k] += seg_counts_list[k][i]
            i += 1
        assert i > d0, "single dst segment exceeds BLK edges"
        blocks.append((d0, i - d0))
    return blocks


def prep(inputs):
    h0 = np.asarray(inputs['h0'], np.float32)
    h1 = np.asarray(inputs['h1'], np.float32)
    Wk = np.asarray(inputs['Wk'], np.float32)
    bk = np.asarray(inputs['bk'], np.float32)
    Wq = np.asarray(inputs['Wq'], np.float32)
    bq = np.asarray(inputs['bq'], np.float32)
    Wv = np.asarray(inputs['Wv'], np.float32)
    bv = np.asarray(inputs['bv'], np.float32)
    Wa = np.asarray(inputs['Wa'], np.float32)
    ba = np.asarray(inputs['ba'], np.float32)
    rel_att = np.asarray(inputs['rel_att'], np.float32)
    rel_msg = np.asarray(inputs['rel_msg'], np.float32)
    rel_pri = np.asarray(inputs['rel_pri'], np.float32)
    skip = np.asarray(inputs['skip'], np.float32)

    alpha = 1.0 / (1.0 + np.exp(-skip))          # [2]
    hs = [h0, h1]

    # effective projections (att/msg/pri folded); bk dropped (cancels in the
    # per-dst softmax). The 0.5 cross-relation mean is folded into Wv/bv of
    # relations 0 and 2.
    Wk_eff, Wv_eff, bv_eff = [], [], []
    for r in range(3):
        st = SRC_OF_REL[r]
        half = 0.5 if r != 1 else 1.0
        A = _block_diag(rel_att[r])
        M = _block_diag(rel_msg[r])
        scale = np.repeat(rel_pri[r] / math.sqrt(DK), DK)  # [256]
        Wk_eff.append((Wk[st] @ A) * scale[None, :])
        Wv_eff.append((Wv[st] @ M) * half)
        bv_eff.append((bv[st] @ M) * half)

    # edge sorting by dst
    edges = []
    for r, (skey, dkey) in enumerate((('src0', 'dst0'), ('src1', 'dst1'),
                                      ('src2', 'dst2'))):
        src = np.asarray(inputs[skey], np.int64)
        dst = np.asarray(inputs[dkey], np.int64)
        order = np.argsort(dst, kind='stable')
        ssrc = src[order]
        sdst = dst[order]
        counts = np.bincount(dst, minlength=N)
        starts = np.zeros(N + 1, np.int64)
        np.cumsum(counts, out=starts[1:])
        edges.append((ssrc, sdst, counts, starts))

    # per-core packing
    per_core = []
    for c in range(NCORES):
        lo = c * RPC
        cnt1 = edges[1][2][lo:lo + RPC]
        blocks0 = _pack_blocks([cnt1])                       # n0 side (rel1)
        cnt0 = edges[0][2][lo:lo + RPC]
        cnt2 = edges[2][2][lo:lo + RPC]
        blocks1 = _pack_blocks([cnt0, cnt2])                 # n1 side (rel0+2)
        per_core.append((blocks0, blocks1))

    NB0 = max(len(pc[0]) for pc in per_core)
    NB1 = max(len(pc[1]) for pc in per_core)
    NB0 += NB0 % 2
    NB1 += NB1 % 2

    # bf16 replicated tables (padded rows are zero)
    h0b = np.zeros((NPAD, D), BF16)
    h0b[:N] = h0.astype(BF16)
    h1b = np.zeros((NPAD, D), BF16)
    h1b[:N] = h1.astype(BF16)

    # weights, chunked for matmul rhs
    wq_t = np.stack([Wq[t].reshape(2, 128, D) for t in range(2)]).astype(BF16)
    wkv_t = np.stack([
        np.concatenate([Wk_eff[r], Wv_eff[r]], axis=1).reshape(2, 128, 2 * D)
        for r in range(3)]).astype(BF16)
    wa_t = np.stack([Wa[t].reshape(2, 128, D) for t in range(2)]).astype(BF16)
    bq_t = bq.copy()                                          # [2, 256] f32
    bv_t = np.stack([bv_eff[1], bv_eff[0] + bv_eff[2]])       # [2, 256]
    use_bv = bool(np.abs(bv_t).max() > 0)

    in_maps = []
    unpack = []
    for c in range(NCORES):
        lo = c * RPC
        blocks0, blocks1 = per_core[c]
        m = {
            'h0b': h0b, 'h1b': h1b,
            'hq0': h0b[lo:lo + QTR].copy(), 'hq1': h1b[lo:lo + QTR].copy(),
            'wq': wq_t, 'wkv': wkv_t, 'wa': wa_t,
            'bq2': bq_t, 'bvt': bv_t.astype(np.float32),
        }
        # per-block q-window offsets (d_lo), side0 blocks then side1 blocks
        dlo = np.zeros((1, NB0 + NB1), np.int32)
        for i, (d_lo, nd) in enumerate(blocks0):
            dlo[0, i] = d_lo
        for i, (d_lo, nd) in enumerate(blocks1):
            dlo[0, NB0 + i] = d_lo
        m['dlo'] = dlo

        # per relation edge data
        rel_blocks = {0: blocks1, 1: blocks0, 2: blocks1}
        rel_nb = {0: NB1, 1: NB0, 2: NB1}
        for r in range(3):
            ssrc, sdst, counts, starts = edges[r]
            blocks = rel_blocks[r]
            nb = rel_nb[r]
            sidx = np.zeros((nb * BLK,), np.int64)
            st = np.zeros((nb, 128, CPB * 128), FP8)
            stT = np.zeros((nb, 128, CPB * 128), FP8)
            for b, (d_lo, nd) in enumerate(blocks):
                e0 = starts[lo + d_lo]
                e1 = starts[lo + d_lo + nd]
                ne = e1 - e0
                assert ne <= BLK
                if ne == 0:
                    continue
                sidx[b * BLK: b * BLK + ne] = ssrc[e0:e1]
                slot = (sdst[e0:e1] - (lo + d_lo)).astype(np.int64)
                j = np.arange(ne)
                st[b, j % 128, (j // 128) * 128 + slot] = 1.0
                stT[b, slot, (j // 128) * 128 + (j % 128)] = 1.0
            groups = [sidx[g * GIDX:(g + 1) * GIDX]
                      for g in range(nb // GRP)]
            m[f'sidx{r}'] = _wrap_idx(groups)
            m[f'st{r}'] = st
            m[f'stT{r}'] = stT

        # skip rows, packed; pre-scaled: h*(1-a) + a*ba
        hsk = np.zeros(((NB0 + NB1) * 128, D), BF16)
        for i, (d_lo, nd) in enumerate(blocks0):
            hsk[i * 128: i * 128 + nd] = (hs[0][lo + d_lo: lo + d_lo + nd]
                                          * (1 - alpha[0]) + alpha[0] * ba[0])
        for i, (d_lo, nd) in enumerate(blocks1):
            hsk[(NB0 + i) * 128:(NB0 + i) * 128 + nd] = (
                hs[1][lo + d_lo: lo + d_lo + nd] * (1 - alpha[1])
                + alpha[1] * ba[1])
        m['hsk'] = hsk
        in_maps.append(m)
        unpack.append((blocks0, blocks1))

    # per-block-index qtab write coverage (max over cores): the qb DMA of
    # block b only needs qtab rows < (jm[b]+1)*128 to be written.
    jm0 = [0] * NB0
    jm1 = [0] * NB1
    for c in range(NCORES):
        blocks0, blocks1 = per_core[c]
        for i, (d_lo, nd) in enumerate(blocks0):
            jm0[i] = max(jm0[i], (d_lo + 127) // 128)
        for i, (d_lo, nd) in enumerate(blocks1):
            jm1[i] = max(jm1[i], (d_lo + 127) // 128)
    meta = dict(NB0=NB0, NB1=NB1, alpha=(float(alpha[0]), float(alpha[1])),
                use_bv=use_bv, jm0=tuple(jm0), jm1=tuple(jm1))
    return in_maps, unpack, meta


# ----------------------------------------------------------------------------
# Device program
# ----------------------------------------------------------------------------

def build_program(NB0, NB1, alpha, use_bv, jm0, jm1):
    fp32 = mybir.dt.float32
    bf16 = mybir.dt.bfloat16
    fp8 = mybir.dt.float8e4
    i16 = mybir.dt.int16
    i32 = mybir.dt.int32
    nc = bacc.Bacc('TRN2', target_bir_lowering=False, debug=False,
                   num_devices=NCORES,
                   dynamic_dma_scratch_size=65536)

    h0b = nc.dram_tensor('h0b', [NPAD, D], bf16, kind='ExternalInput')
    h1b = nc.dram_tensor('h1b', [NPAD, D], bf16, kind='ExternalInput')
    hq = [nc.dram_tensor(f'hq{t}', [QTR, D], bf16, kind='ExternalInput')
          for t in range(2)]
    wq = nc.dram_tensor('wq', [2, 2, 128, D], bf16, kind='ExternalInput')
    wkv = nc.dram_tensor('wkv', [3, 2, 128, 2 * D], bf16, kind='ExternalInput')
    wa = nc.dram_tensor('wa', [2, 2, 128, D], bf16, kind='ExternalInput')
    bq2 = nc.dram_tensor('bq2', [2, D], fp32, kind='ExternalInput')
    bvt = nc.dram_tensor('bvt', [2, D], fp32, kind='ExternalInput')
    dlo_t = nc.dram_tensor('dlo', [1, NB0 + NB1], i32, kind='ExternalInput')
    sidx = [nc.dram_tensor(f'sidx{r}', [128, (NB1 if r != 1 else NB0) * ICOLS],
                           i16, kind='ExternalInput') for r in range(3)]
    st = [nc.dram_tensor(f'st{r}', [(NB1 if r != 1 else NB0), 128, CPB * 128],
                         fp8, kind='ExternalInput') for r in range(3)]
    stT = [nc.dram_tensor(f'stT{r}', [(NB1 if r != 1 else NB0), 128,
                                      CPB * 128],
                          fp8, kind='ExternalInput') for r in range(3)]
    hsk = nc.dram_tensor('hsk', [(NB0 + NB1) * 128, D], bf16,
                         kind='ExternalInput')
    outp = nc.dram_tensor('outp', [(NB0 + NB1) * 128, D], bf16,
                          kind='ExternalOutput')
    qtab = [nc.dram_tensor(f'q{t}tab', [QTR, D], bf16, kind='Internal')
            for t in range(2)]

    htab = [h0b, h1b]
    REL_NB = {0: NB1, 1: NB0, 2: NB1}

    with tile.TileContext(nc) as tc:
        with (
            tc.tile_pool(name='singles', bufs=1) as singles,
            tc.tile_pool(name='gpool', bufs=10) as gpool,
            tc.tile_pool(name='spool', bufs=4) as spool,
            tc.tile_pool(name='work', bufs=8) as work,
            tc.tile_pool(name='opool', bufs=3) as opool,
            tc.tile_pool(name='psA', bufs=2, space='PSUM') as psA,
            tc.tile_pool(name='psQ', bufs=2, space='PSUM') as psQ,
            tc.tile_pool(name='psU', bufs=1, space='PSUM') as psU,
            tc.tile_pool(name='psO', bufs=1, space='PSUM') as psO,
        ):
            from concourse import library_config
            from concourse.tile import add_dep_helper
            nc.gpsimd.load_library(library_config.mlp)
            gidx_reg = nc.gpsimd.to_reg(GIDX)
            qdma_hist = []

            ident = singles.tile([128, 128], bf16)
            make_identity(nc, ident[:])

            # resident weights
            wq_sb = singles.tile([128, 2, 2, D], bf16)
            nc.sync.dma_start(out=wq_sb[:],
                              in_=wq[:].rearrange('a b p n -> p a b n'))
            wkv_sb = singles.tile([128, 3, 2, 2 * D], bf16)
            nc.sync.dma_start(out=wkv_sb[:],
                              in_=wkv[:].rearrange('a b p n -> p a b n'))
            wa_sb = singles.tile([128, 2, 2, D], bf16)
            nc.sync.dma_start(out=wa_sb[:],
                              in_=wa[:].rearrange('a b p n -> p a b n'))
            bq_sb = singles.tile([128, 2, D], fp32)
            bv_sb = singles.tile([128, 2, D], fp32)
            for t in range(2):
                src = bq2[t:t + 1, :]
                nc.sync.dma_start(out=bq_sb[:, t, :],
                                  in_=src.to_broadcast([128, D]))
                src = bvt[t:t + 1, :]
                nc.sync.dma_start(out=bv_sb[:, t, :],
                                  in_=src.to_broadcast([128, D]))

            # resident indices + per-block q-window offsets
            sidx_sb = []
            for r in range(3):
                t1 = singles.tile([128, REL_NB[r] * ICOLS], i16,
                                  tag=f'sidx{r}')
                nc.sync.dma_start(out=t1[:], in_=sidx[r][:])
                sidx_sb.append(t1)
            dlo_sb = singles.tile([1, NB0 + NB1], i32, tag='dlo')
            nc.sync.dma_start(out=dlo_sb[:], in_=dlo_t[:])

            # t-store for rel0 results (n1 side)
            tstore = singles.tile([128, NB1, D], bf16)

            # own h rows, DMA-transposed (lhsT layout) for the q projection
            gth_t = []
            for t in range(2):
                g = singles.tile([128, 2, QTR], bf16, tag=f'gth{t}')
                for cc in range(2):
                    nc.scalar.dma_start_transpose(
                        out=g[:, cc, :],
                        in_=hq[t][:, cc * 128:(cc + 1) * 128])
                gth_t.append(g)

            # ---------------- phase: q tables ----------------
            # No barrier afterwards: each per-block qb DMA carries explicit
            # dep edges on this side's qtab writes. Alternate psA/psQ tiles
            # so the projection pipeline is >1 deep.
            qtab_dmas = [[], []]
            for t in range(2):
                for j in range(QTR // 128):
                    if j % 2 == 0:
                        qp = psA.tile([128, 2, 2 * D], fp32, tag='kv')
                    else:
                        qp = psQ.tile([128, 2, D], fp32, tag='qg')
                    for cc in range(2):
                        nc.tensor.matmul(
                            out=qp[:, 0, :D],
                            lhsT=gth_t[t][:, cc, j * 128:(j + 1) * 128],
                            rhs=wq_sb[:, t, cc, :],
                            start=(cc == 0), stop=(cc == 1))
                    qs = work.tile([128, D], bf16, tag='qs')
                    nc.vector.tensor_add(qs[:], qp[:, 0, :D],
                                         bq_sb[:, t, :])
                    dma = nc.sync.dma_start(
                        out=qtab[t][j * 128:(j + 1) * 128, :], in_=qs[:])
                    qtab_dmas[t].append(dma)

            # ---------------- relation passes ----------------
            def rel_pass(r, mode):
                # mode: 'out' (rel1), 'store' (rel0), 'combine' (rel2)
                nb = REL_NB[r]
                side = DST_OF_REL[r]
                out_off = 0 if side == 0 else NB0
                scol = 0 if side == 0 else NB0
                for g in range(nb // GRP):
                    sg = gpool.tile([128, 2, GIDX], bf16, tag='gs')
                    nc.gpsimd.dma_gather(
                        out_ap=sg[:], in_ap=htab[SRC_OF_REL[r]][:],
                        idxs_ap=sidx_sb[r][:, g * (GIDX // 16):
                                           (g + 1) * (GIDX // 16)],
                        num_idxs=GIDX, num_idxs_reg=gidx_reg,
                        elem_size=D, transpose=True)
                    for bb in range(GRP):
                        b = g * GRP + bb
                        # block q rows (d_lo..d_lo+127) via dynamic-offset
                        # DMA; an explicit dep on the 4-back qb DMA bounds
                        # sync-engine register liveness.
                        dreg = nc.sync.alloc_register(f'dlo_{r}_{b}')
                        ld = nc.sync.reg_load(
                            dreg, dlo_sb[0:1, scol + b: scol + b + 1])
                        if len(qdma_hist) >= 4:
                            add_dep_helper(ld.ins, qdma_hist[-4].ins,
                                           sync=True,
                                           reason='bound dlo reg liveness')
                        dval = nc.sync.snap(dreg, donate=True, min_val=0,
                                            max_val=QTR - 128)
                        qb = opool.tile([128, D], bf16, tag='qb')
                        qdma = nc.sync.dma_start(
                            out=qb[:], in_=qtab[side][bass.ds(dval, 128), :])
                        jm = (jm0 if side == 0 else jm1)[b]
                        for wdma in qtab_dmas[side][:jm + 1]:
                            add_dep_helper(qdma.ins, wdma.ins, sync=True,
                                           reason='qtab RAW')
                        qdma_hist.append(qdma)
                        stt = spool.tile([128, CPB * 128], fp8, tag='st')
                        nc.sync.dma_start(out=stt[:], in_=st[r][b])
                        sttT = spool.tile([128, CPB * 128], fp8, tag='stT')
                        nc.sync.dma_start(out=sttT[:], in_=stT[r][b])
                        u = psU.tile([128, 264], fp32, tag='u')
                        for pair in ((0, 1), (2, 3), (4,)):
                            npair = len(pair)
                            kv = psA.tile([128, 2, 2 * D], fp32, tag='kv')
                            qg_ps = psQ.tile([128, 2, D], fp32, tag='qg')
                            for i, j in enumerate(pair):
                                ci = bb * CPB + j
                                for cc in range(2):
                                    nc.tensor.matmul(
                                        out=kv[:, i, :],
                                        lhsT=sg[:, cc,
                                                ci * 128:(ci + 1) * 128],
                                        rhs=wkv_sb[:, r, cc, :],
                                        start=(cc == 0), stop=(cc == 1))
                                nc.tensor.matmul(
                                    out=qg_ps[:, i, :],
                                    lhsT=sttT[:, j * 128:(j + 1) * 128],
                                    rhs=qb[:], start=True, stop=True)
                            qg = work.tile([128, 2, D], bf16, tag='qgs')
                            nc.scalar.activation(
                                qg[:, :npair, :], qg_ps[:, :npair, :],
                                mybir.ActivationFunctionType.Copy)
                            p = work.tile([128, 2, D], bf16, tag='p')
                            nc.vector.tensor_mul(p[:, :npair, :],
                                                 qg[:, :npair, :],
                                                 kv[:, :npair, :D])
                            s8 = work.tile([128, 2, H], fp32, tag='s8')
                            nc.vector.reduce_sum(
                                s8[:, :npair, :],
                                p[:, :npair, :].rearrange(
                                    'p a (h d) -> p a h d', d=DK),
                                axis=mybir.AxisListType.X)
                            ex = work.tile([128, 2, H], fp32, tag='ex')
                            nc.scalar.activation(
                                ex[:, :npair, :], s8[:, :npair, :],
                                mybir.ActivationFunctionType.Exp)
                            rhs = work.tile([128, 2, 264], bf16, tag='rhs')
                            exb = ex[:, :npair, :]
                            exb = bass.AP(tensor=exb.tensor, offset=exb.offset,
                                          ap=[*exb.ap, [0, DK]])
                            nc.vector.tensor_mul(
                                rhs[:, :npair, :D].rearrange(
                                    'p a (h d) -> p a h d', d=DK),
                                kv[:, :npair, D:].rearrange(
                                    'p a (h d) -> p a h d', d=DK),
                                exb)
                            nc.scalar.activation(
                                rhs[:, :npair, D:D + H], ex[:, :npair, :],
                                mybir.ActivationFunctionType.Copy)
                            for i, j in enumerate(pair):
                                nc.tensor.matmul(
                                    out=u[:],
                                    lhsT=stt[:, j * 128:(j + 1) * 128],
                                    rhs=rhs[:, i, :],
                                    start=(j == 0), stop=(j == CPB - 1))
                        # normalize
                        rcp = work.tile([128, H], fp32, tag='rcp')
                        nc.vector.tensor_scalar_add(rcp[:], u[:, D:D + H],
                                                    1e-20)
                        nc.vector.reciprocal(rcp[:], rcp[:])
                        rcpb = rcp[:]
                        rcpb = bass.AP(tensor=rcpb.tensor, offset=rcpb.offset,
                                       ap=[*rcpb.ap, [0, DK]])
                        if mode == 'store':
                            nc.vector.tensor_mul(
                                tstore[:, b, :].rearrange(
                                    'p (h d) -> p h d', d=DK),
                                u[:, :D].rearrange('p (h d) -> p h d', d=DK),
                                rcpb)
                            continue
                        t_sb = opool.tile([128, D], bf16, tag='t')
                        nc.vector.tensor_mul(
                            t_sb[:].rearrange('p (h d) -> p h d', d=DK),
                            u[:, :D].rearrange('p (h d) -> p h d', d=DK),
                            rcpb)
                        if mode == 'combine':
                            nc.vector.tensor_add(t_sb[:], t_sb[:],
                                                 tstore[:, b, :])
                        if use_bv:
                            nc.vector.tensor_add(t_sb[:], t_sb[:],
                                                 bv_sb[:, side, :])
                        # output: transpose, matmul Wa, skip-combine. om and
                        # the transpose scratch share one PSUM bank.
                        tts = opool.tile([128, 2, 128], bf16, tag='tts')
                        omtp = psO.tile([128, 384], fp32, tag='omtp')
                        om = omtp[:, :D]
                        tp = omtp[:, D:D + 64].bitcast(bf16)
                        for cc in range(2):
                            nc.tensor.transpose(
                                tp, t_sb[:, cc * 128:(cc + 1) * 128],
                                ident[:])
                            nc.scalar.activation(
                                tts[:, cc, :], tp,
                                mybir.ActivationFunctionType.Copy)
                        for cc in range(2):
                            nc.tensor.matmul(
                                out=om, lhsT=tts[:, cc, :],
                                rhs=wa_sb[:, side, cc, :],
                                start=(cc == 0), stop=(cc == 1))
                        hs_t = opool.tile([128, D], bf16, tag='hs')
                        row = (out_off + b) * 128
                        nc.sync.dma_start(out=hs_t[:],
                                          in_=hsk[row:row + 128, :])
                        o_sb = opool.tile([128, D], bf16, tag='o')
                        nc.vector.scalar_tensor_tensor(
                            out=o_sb[:], in0=om, scalar=alpha[side],
                            in1=hs_t[:], op0=mybir.AluOpType.mult,
                            op1=mybir.AluOpType.add)
                        nc.sync.dma_start(out=outp[row:row + 128, :],
                                          in_=o_sb[:])

            rel_pass(1, 'out')
            rel_pass(0, 'store')
            rel_pass(2, 'combine')

    nc.compile()
    return nc


# ----------------------------------------------------------------------------
# Entry point
# ----------------------------------------------------------------------------

def _run(inputs, trace=False):
    in_maps, unpack, meta = prep(inputs)
    key = (meta['NB0'], meta['NB1'], meta['alpha'], meta['use_bv'],
           meta['jm0'], meta['jm1'])
    if key not in _cache:
        _cache[key] = build_program(meta['NB0'], meta['NB1'], meta['alpha'],
                                    meta['use_bv'], meta['jm0'], meta['jm1'])
    nc = _cache[key]
    res = run_bass_kernel_spmd(nc, in_maps, core_ids=list(range(NCORES)),
                               trace=trace)
    NB0 = meta['NB0']
    out = np.zeros((2, N, D), np.float32)
    for c in range(NCORES):
        lo = c * RPC
        op = res.results[c]['outp']
        blocks0, blocks1 = unpack[c]
        for i, (d_lo, nd) in enumerate(blocks0):
            out[0, lo + d_lo: lo + d_lo + nd] = op[i * 128: i * 128 + nd]
        for i, (d_lo, nd) in enumerate(blocks1):
            out[1, lo + d_lo: lo + d_lo + nd] = op[(NB0 + i) * 128:
                                                   (NB0 + i) * 128 + nd]
    return out, res


def kernel(**inputs):
    out, _ = _run(inputs, trace=False)
    return out
